# revision 18
# baseline (speedup 1.0000x reference)
"""Banded local attention on 8 Trainium2 NeuronCores (Bass/Tile).

Problem: B=2, L=2048, H=8, E=64, band |i-j| <= w with w = ceil(1.2*log2(L)/2) = 7.

Sharding: 16 (batch, head) units across 8 cores, 2 units per core.
Each core computes its two heads' banded attention fully independently.

Per-head algorithm (18 query tiles of 114 queries):
  For query tile [q0, q0+114) the band only touches keys [q0-7, q0+121), which
  fits a single 128-key window [k0, k0+128).  Scores are computed transposed,
  ST[k, q] = K_win @ Q_tile^T, in bf16 with e on partitions; the K=64
  contraction means the two heads (SBUF partition bases 0 and 64) run on
  disjoint PE row-groups and pipeline tightly.  exp(ST/8) on ScalarE (no max
  subtraction: unit-scale inputs can't overflow exp; softmax is
  shift-invariant).  Multiply by the 0/1 band mask (out-of-band -> exactly 0,
  matching exp(-inf)).  One matmul with V_aug = [V_win | 1] as stationary gives
  OT[65, q] = [unnormalized out^T; denominator row].  PE-transpose OT,
  reciprocal of the denominator column and a per-partition tensor_scalar
  multiply produce the normalized [q, 64] output tile, accumulated in an SBUF
  buffer and stored with one large DMA per head.  Both heads share every
  elementwise op: their PSUM tiles live in adjacent banks of one two-bank tile
  ([*, 2, 512] fp32), so ScalarE/VectorE see a single strided AP.
"""

import ml_dtypes
import numpy as np

import concourse.bass as bass
import concourse.tile as tile
from concourse import bacc, mybir
from concourse.bass_utils import run_bass_kernel_spmd

B, L, H, E = 2, 2048, 8, 64
W = 7
NCORES = 8
QT = 114  # queries per tile
KW = 128  # key window per tile
NT = 18  # tiles per head
HPC = 2  # heads (b,h units) per core
PSB = 512  # fp32 elements per PSUM bank
F32 = mybir.dt.float32
BF16 = mybir.dt.bfloat16

EXP = mybir.ActivationFunctionType.Exp


def _tile_params():
    params = []
    for t in range(NT):
        q0 = t * QT if t < NT - 1 else L - QT
        if t == 0:
            k0 = 0
        elif t < NT - 1:
            k0 = t * QT - W
        else:
            k0 = L - KW
        mid = 0 if t == 0 else (1 if t < NT - 1 else 2)
        so = 0 if t < NT - 1 else (NT - 1) * QT - q0  # rows already stored by t-1
        params.append((q0, k0, mid, so))
    return params


_PARAMS = _tile_params()


def _build_masks():
    # mask[p, m, h, j] = 1.0 iff |(k0-q0)_m + p - j| <= W (duplicated per head)
    deltas = [0, -W, -(2 * W)]
    p = np.arange(KW)[:, None]
    j = np.arange(QT)[None, :]
    m = np.stack([(np.abs(d + p - j) <= W) for d in deltas], axis=1)  # [128,3,114]
    m = np.repeat(m[:, :, None, :], HPC, axis=2)  # [128, 3, 2, 114]
    return np.ascontiguousarray(m.astype(ml_dtypes.bfloat16))


def _build_program():
    nc = bacc.Bacc("TRN2", target_bir_lowering=False, debug=False, enable_partition_id=False)

    qh_d = nc.dram_tensor("qh", [128, L], BF16, kind="ExternalInput")
    kh_d = nc.dram_tensor("kh", [128, L], BF16, kind="ExternalInput")
    vw_d = nc.dram_tensor("vw", [128, HPC, NT, E + 1], BF16, kind="ExternalInput")
    mk_d = nc.dram_tensor("mk", [128, 3, HPC, QT], BF16, kind="ExternalInput")
    out_d = nc.dram_tensor("o", [HPC, L, E], F32, kind="ExternalOutput")

    with tile.TileContext(nc) as tc:
        with (
            tc.tile_pool(name="const", bufs=1) as cpool,
            tc.tile_pool(name="work", bufs=6) as work,
            tc.tile_pool(name="ps", bufs=2, space="PSUM") as ps,
            tc.tile_pool(name="ps1", bufs=2, space="PSUM") as ps1,
        ):
            qh_s = cpool.tile([128, L], BF16)
            kh_s = cpool.tile([128, L], BF16)
            vw_s = cpool.tile([128, HPC, NT, E + 1], BF16)
            mk_s = cpool.tile([128, 3, HPC, QT], BF16)
            # small first slices on sync so tile 0/1 deps land early;
            # bulk streams on scalar in parallel
            FS = 256
            nc.sync.dma_start(mk_s[:], mk_d.ap()[:])
            nc.sync.dma_start(kh_s[:, 0:FS], kh_d.ap()[:, 0:FS])
            nc.sync.dma_start(qh_s[:, 0:FS], qh_d.ap()[:, 0:FS])
            nc.sync.dma_start(vw_s[:, :, 0:3, :], vw_d.ap()[:, :, 0:3, :])
            nc.scalar.dma_start(kh_s[:, FS:L], kh_d.ap()[:, FS:L])
            nc.scalar.dma_start(qh_s[:, FS:L], qh_d.ap()[:, FS:L])
            nc.scalar.dma_start(vw_s[:, :, 3:NT, :], vw_d.ap()[:, :, 3:NT, :])
            obuf = cpool.tile([QT, HPC, NT - 1, E], F32)
            # warm the exp table while DMAs stream
            dum = work.tile([1, 1], F32, tag="dum")
            nc.scalar.activation(dum[:], dum[:], EXP)

            for t in range(NT):
                q0, k0, mid, so = _PARAMS[t]
                kwin = slice(k0, k0 + KW)
                qwin = slice(q0, q0 + QT)
                # scores^T per head into adjacent PSUM banks
                st = ps.tile([KW, HPC, PSB], F32, tag="st")
                for h in range(HPC):
                    hp = h * E
                    nc.tensor.matmul(
                        st[:, h, 0:QT],
                        kh_s[hp : hp + E, kwin],
                        qh_s[hp : hp + E, qwin],
                    )
                # exp(scores/8), both heads in one op
                ex = work.tile([KW, HPC, QT], BF16, tag="ex")
                nc.scalar.activation(ex[:], st[:, :, 0:QT], EXP, scale=1.0 / 8.0)
                # band mask (0/1 multiply), both heads (mask identical per head)
                at = work.tile([KW, HPC, QT], BF16, tag="at")
                nc.vector.tensor_mul(at[:], ex[:], mk_s[:, mid, :, :])
                # attn^T @ [V_win | 1] -> [q, 65]: cols 0..63 out, col 64 denom
                o = ps1.tile([QT, HPC, PSB], F32, tag="o")
                for h in range(HPC):
                    nc.tensor.matmul(o[:, h, 0 : E + 1], at[:, h, :], vw_s[:, h, t, :])
                # normalize: out[q, e] = o[q, e] / o[q, 64], both heads at once
                rc = work.tile([QT, HPC], F32, tag="rc")
                nc.vector.reciprocal(rc[:], o[:, :, E])
                if t < NT - 1:
                    nc.vector.tensor_tensor(
                        obuf[:, :, t, :],
                        o[:, :, 0:E],
                        rc[:].broadcast_to([QT, HPC, E]),
                        mybir.AluOpType.mult,
                    )
                else:
                    for h in range(HPC):
                        oo = work.tile([QT, E], F32, tag=f"oo{h}")
                        nc.vector.tensor_scalar_mul(
                            oo[:], o[:, h, 0:E], rc[:, h : h + 1]
                        )
                        (nc.sync if h == 0 else nc.scalar).dma_start(
                            out_d.ap()[h, q0 + so : L, :], oo[so:QT, :]
                        )
                if t in (3, 7, 11, 13, 15, 16):
                    c0 = {3: 0, 7: 4, 11: 8, 13: 12, 15: 14, 16: 16}[t]
                    c1 = t + 1
                    for h in range(HPC):
                        eng = nc.sync if h == 0 else nc.scalar
                        eng.dma_start(
                            out_d.ap()[h, c0 * QT : c1 * QT, :].rearrange(
                                "(t p) e -> p t e", p=QT
                            ),
                            obuf[:, h, c0:c1, :],
                        )

    nc.compile()
    return nc


_NC_CACHE = None


def _get_program():
    global _NC_CACHE
    if _NC_CACHE is None:
        _NC_CACHE = _build_program()
    return _NC_CACHE


def _core_inputs(queries, keys, values, c, masks):
    bf = ml_dtypes.bfloat16
    qt = np.empty((128, L), dtype=np.float32)
    kt = np.empty((128, L), dtype=np.float32)
    vw = np.ones((128, HPC, NT, E + 1), dtype=bf)
    k0s = np.array([p[1] for p in _PARAMS])  # [NT]
    rows = k0s[:, None] + np.arange(KW)[None, :]  # [NT, 128]
    for j in range(HPC):
        u = HPC * c + j
        b, h = divmod(u, H)
        qt[E * j : E * (j + 1)] = queries[b, :, h, :].T
        kt[E * j : E * (j + 1)] = keys[b, :, h, :].T
        vh = values[b, :, h, :]  # [L, E]
        vw[:, j, :, :E] = vh[rows].transpose(1, 0, 2).astype(bf)
    return {
        "qh": qt.astype(bf),
        "kh": kt.astype(bf),
        "vw": vw,
        "mk": masks,
    }


def _run(queries, keys, values, trace=False):
    nc = _get_program()
    masks = _build_masks()
    in_maps = [
        _core_inputs(queries, keys, values, c, masks) for c in range(NCORES)
    ]
    res = run_bass_kernel_spmd(nc, in_maps, list(range(NCORES)), trace=trace)
    out = np.empty((B, L, H, E), dtype=np.float32)
    for c in range(NCORES):
        o = res.results[c]["o"]
        for j in range(HPC):
            u = HPC * c + j
            b, h = divmod(u, H)
            out[b, :, h, :] = o[j]
    return out, res


def kernel(queries, keys, values):
    out, _ = _run(
        np.asarray(queries, dtype=np.float32),
        np.asarray(keys, dtype=np.float32),
        np.asarray(values, dtype=np.float32),
    )
    return out


# revision 20
# speedup vs baseline: 1.0041x; 1.0041x over previous
"""Banded local attention on 8 Trainium2 NeuronCores (Bass/Tile).

Problem: B=2, L=2048, H=8, E=64, band |i-j| <= w with w = ceil(1.2*log2(L)/2) = 7.

Sharding: 16 (batch, head) units across 8 cores, 2 units per core.
Each core computes its two heads' banded attention fully independently.

Per-head algorithm (18 query tiles of 114 queries):
  For query tile [q0, q0+114) the band only touches keys [q0-7, q0+121), which
  fits a single 128-key window [k0, k0+128).  Scores are computed transposed,
  ST[k, q] = K_win @ Q_tile^T, in bf16 with e on partitions; the K=64
  contraction means the two heads (SBUF partition bases 0 and 64) run on
  disjoint PE row-groups and pipeline tightly.  exp(ST/8) on ScalarE (no max
  subtraction: unit-scale inputs can't overflow exp; softmax is
  shift-invariant).  Multiply by the 0/1 band mask (out-of-band -> exactly 0,
  matching exp(-inf)).  One matmul with V_aug = [V_win | 1] as stationary gives
  OT[65, q] = [unnormalized out^T; denominator row].  PE-transpose OT,
  reciprocal of the denominator column and a per-partition tensor_scalar
  multiply produce the normalized [q, 64] output tile, accumulated in an SBUF
  buffer and stored with one large DMA per head.  Both heads share every
  elementwise op: their PSUM tiles live in adjacent banks of one two-bank tile
  ([*, 2, 512] fp32), so ScalarE/VectorE see a single strided AP.
"""

import ml_dtypes
import numpy as np

import concourse.bass as bass
import concourse.tile as tile
from concourse import bacc, mybir
from concourse.bass_utils import run_bass_kernel_spmd

B, L, H, E = 2, 2048, 8, 64
W = 7
NCORES = 8
QT = 114  # queries per tile
KW = 128  # key window per tile
NT = 18  # tiles per head
HPC = 2  # heads (b,h units) per core
PSB = 512  # fp32 elements per PSUM bank
F32 = mybir.dt.float32
BF16 = mybir.dt.bfloat16

EXP = mybir.ActivationFunctionType.Exp


def _tile_params():
    params = []
    for t in range(NT):
        q0 = t * QT if t < NT - 1 else L - QT
        if t == 0:
            k0 = 0
        elif t < NT - 1:
            k0 = t * QT - W
        else:
            k0 = L - KW
        mid = 0 if t == 0 else (1 if t < NT - 1 else 2)
        so = 0 if t < NT - 1 else (NT - 1) * QT - q0  # rows already stored by t-1
        params.append((q0, k0, mid, so))
    return params


_PARAMS = _tile_params()


def _build_masks():
    # mask[p, m, h, j] = 1.0 iff |(k0-q0)_m + p - j| <= W (duplicated per head)
    deltas = [0, -W, -(2 * W)]
    p = np.arange(KW)[:, None]
    j = np.arange(QT)[None, :]
    m = np.stack([(np.abs(d + p - j) <= W) for d in deltas], axis=1)  # [128,3,114]
    m = np.repeat(m[:, :, None, :], HPC, axis=2)  # [128, 3, 2, 114]
    return np.ascontiguousarray(m.astype(ml_dtypes.bfloat16))


def _build_program():
    nc = bacc.Bacc("TRN2", target_bir_lowering=False, debug=False, enable_partition_id=False, enable_asserts=False)

    qh_d = nc.dram_tensor("qh", [128, L], BF16, kind="ExternalInput")
    kh_d = nc.dram_tensor("kh", [128, L], BF16, kind="ExternalInput")
    vw_d = nc.dram_tensor("vw", [128, HPC, NT, E + 1], BF16, kind="ExternalInput")
    mk_d = nc.dram_tensor("mk", [128, 3, HPC, QT], BF16, kind="ExternalInput")
    out_d = nc.dram_tensor("o", [HPC, L, E], F32, kind="ExternalOutput")

    with tile.TileContext(nc) as tc:
        with (
            tc.tile_pool(name="const", bufs=1) as cpool,
            tc.tile_pool(name="work", bufs=4) as work,
            tc.tile_pool(name="ps", bufs=2, space="PSUM") as ps,
            tc.tile_pool(name="ps1", bufs=2, space="PSUM") as ps1,
        ):
            qh_s = cpool.tile([128, L], BF16)
            kh_s = cpool.tile([128, L], BF16)
            vw_s = cpool.tile([128, HPC, NT, E + 1], BF16)
            mk_s = cpool.tile([128, 3, HPC, QT], BF16)
            nc.gpsimd.dma_start(mk_s[:], mk_d.ap()[:])
            nc.gpsimd.dma_start(vw_s[:, :, 0:3, :], vw_d.ap()[:, :, 0:3, :])
            nc.gpsimd.dma_start(vw_s[:, :, 3:NT, :], vw_d.ap()[:, :, 3:NT, :])
            # small first slices so tile 0/1 deps land early, then the rest
            FS = 256
            nc.sync.dma_start(kh_s[:, 0:FS], kh_d.ap()[:, 0:FS])
            nc.sync.dma_start(qh_s[:, 0:FS], qh_d.ap()[:, 0:FS])
            nc.sync.dma_start(kh_s[:, FS:L], kh_d.ap()[:, FS:L])
            nc.sync.dma_start(qh_s[:, FS:L], qh_d.ap()[:, FS:L])
            obuf = cpool.tile([QT, HPC, NT - 1, E], F32)
            # warm the exp table while DMAs stream
            dum = work.tile([1, 1], F32, tag="dum")
            nc.scalar.activation(dum[:], dum[:], EXP)

            for t in range(NT):
                q0, k0, mid, so = _PARAMS[t]
                kwin = slice(k0, k0 + KW)
                qwin = slice(q0, q0 + QT)
                # scores^T per head into adjacent PSUM banks
                st = ps.tile([KW, HPC, PSB], F32, tag="st")
                for h in range(HPC):
                    hp = h * E
                    nc.tensor.matmul(
                        st[:, h, 0:QT],
                        kh_s[hp : hp + E, kwin],
                        qh_s[hp : hp + E, qwin],
                    )
                # exp(scores/8), both heads in one op
                ex = work.tile([KW, HPC, QT], BF16, tag="ex")
                nc.scalar.activation(ex[:], st[:, :, 0:QT], EXP, scale=1.0 / 8.0)
                # band mask (0/1 multiply), both heads (mask identical per head)
                at = work.tile([KW, HPC, QT], BF16, tag="at")
                nc.vector.tensor_mul(at[:], ex[:], mk_s[:, mid, :, :])
                # attn^T @ [V_win | 1] -> [q, 65]: cols 0..63 out, col 64 denom
                o = ps1.tile([QT, HPC, PSB], F32, tag="o")
                for h in range(HPC):
                    nc.tensor.matmul(o[:, h, 0 : E + 1], at[:, h, :], vw_s[:, h, t, :])
                # normalize: out[q, e] = o[q, e] / o[q, 64], both heads at once
                rc = work.tile([QT, HPC], F32, tag="rc")
                nc.vector.reciprocal(rc[:], o[:, :, E])
                if t < NT - 1:
                    nc.vector.tensor_tensor(
                        obuf[:, :, t, :],
                        o[:, :, 0:E],
                        rc[:].broadcast_to([QT, HPC, E]),
                        mybir.AluOpType.mult,
                    )
                else:
                    for h in range(HPC):
                        oo = work.tile([QT, E], F32, tag=f"oo{h}")
                        nc.vector.tensor_scalar_mul(
                            oo[:], o[:, h, 0:E], rc[:, h : h + 1]
                        )
                        nc.sync.dma_start(
                            out_d.ap()[h, q0 + so : L, :], oo[so:QT, :]
                        )
                if t in (3, 7, 11, 14, 16):
                    c0 = {3: 0, 7: 4, 11: 8, 14: 12, 16: 15}[t]
                    c1 = t + 1
                    for h in range(HPC):
                        eng = nc.sync
                        eng.dma_start(
                            out_d.ap()[h, c0 * QT : c1 * QT, :].rearrange(
                                "(t p) e -> p t e", p=QT
                            ),
                            obuf[:, h, c0:c1, :],
                        )

    nc.compile()
    return nc


_NC_CACHE = None


def _get_program():
    global _NC_CACHE
    if _NC_CACHE is None:
        _NC_CACHE = _build_program()
    return _NC_CACHE


def _core_inputs(queries, keys, values, c, masks):
    bf = ml_dtypes.bfloat16
    qt = np.empty((128, L), dtype=np.float32)
    kt = np.empty((128, L), dtype=np.float32)
    vw = np.ones((128, HPC, NT, E + 1), dtype=bf)
    k0s = np.array([p[1] for p in _PARAMS])  # [NT]
    rows = k0s[:, None] + np.arange(KW)[None, :]  # [NT, 128]
    for j in range(HPC):
        u = HPC * c + j
        b, h = divmod(u, H)
        qt[E * j : E * (j + 1)] = queries[b, :, h, :].T
        kt[E * j : E * (j + 1)] = keys[b, :, h, :].T
        vh = values[b, :, h, :]  # [L, E]
        vw[:, j, :, :E] = vh[rows].transpose(1, 0, 2).astype(bf)
    return {
        "qh": qt.astype(bf),
        "kh": kt.astype(bf),
        "vw": vw,
        "mk": masks,
    }


def _run(queries, keys, values, trace=False):
    nc = _get_program()
    masks = _build_masks()
    in_maps = [
        _core_inputs(queries, keys, values, c, masks) for c in range(NCORES)
    ]
    res = run_bass_kernel_spmd(nc, in_maps, list(range(NCORES)), trace=trace)
    out = np.empty((B, L, H, E), dtype=np.float32)
    for c in range(NCORES):
        o = res.results[c]["o"]
        for j in range(HPC):
            u = HPC * c + j
            b, h = divmod(u, H)
            out[b, :, h, :] = o[j]
    return out, res


def kernel(queries, keys, values):
    out, _ = _run(
        np.asarray(queries, dtype=np.float32),
        np.asarray(keys, dtype=np.float32),
        np.asarray(values, dtype=np.float32),
    )
    return out


# revision 21
# speedup vs baseline: 1.0388x; 1.0346x over previous
"""Banded local attention on 8 Trainium2 NeuronCores (Bass/Tile).

Problem: B=2, L=2048, H=8, E=64, band |i-j| <= w with w = ceil(1.2*log2(L)/2) = 7.

Sharding: 16 (batch, head) units across 8 cores, 2 units per core.
Each core computes its two heads' banded attention fully independently.

Per-head algorithm (18 query tiles of 114 queries):
  For query tile [q0, q0+114) the band only touches keys [q0-7, q0+121), which
  fits a single 128-key window [k0, k0+128).  Scores are computed transposed,
  ST[k, q] = K_win @ Q_tile^T, in bf16 with e on partitions; the K=64
  contraction means the two heads (SBUF partition bases 0 and 64) run on
  disjoint PE row-groups and pipeline tightly.  exp(ST/8) on ScalarE (no max
  subtraction: unit-scale inputs can't overflow exp; softmax is
  shift-invariant).  Multiply by the 0/1 band mask (out-of-band -> exactly 0,
  matching exp(-inf)).  One matmul with V_aug = [V_win | 1] as stationary gives
  OT[65, q] = [unnormalized out^T; denominator row].  PE-transpose OT,
  reciprocal of the denominator column and a per-partition tensor_scalar
  multiply produce the normalized [q, 64] output tile, accumulated in an SBUF
  buffer and stored with one large DMA per head.  Both heads share every
  elementwise op: their PSUM tiles live in adjacent banks of one two-bank tile
  ([*, 2, 512] fp32), so ScalarE/VectorE see a single strided AP.
"""

import ml_dtypes
import numpy as np

import concourse.bass as bass
import concourse.tile as tile
from concourse import bacc, mybir
from concourse.bass_utils import run_bass_kernel_spmd

B, L, H, E = 2, 2048, 8, 64
W = 7
NCORES = 8
QT = 114  # queries per tile
KW = 128  # key window per tile
NT = 18  # tiles per head
HPC = 2  # heads (b,h units) per core
PSB = 512  # fp32 elements per PSUM bank
F32 = mybir.dt.float32
BF16 = mybir.dt.bfloat16

EXP = mybir.ActivationFunctionType.Exp


def _tile_params():
    params = []
    for t in range(NT):
        q0 = t * QT if t < NT - 1 else L - QT
        if t == 0:
            k0 = 0
        elif t < NT - 1:
            k0 = t * QT - W
        else:
            k0 = L - KW
        mid = 0 if t == 0 else (1 if t < NT - 1 else 2)
        so = 0 if t < NT - 1 else (NT - 1) * QT - q0  # rows already stored by t-1
        params.append((q0, k0, mid, so))
    return params


_PARAMS = _tile_params()


def _build_masks():
    # mask[p, m, h, j] = 1.0 iff |(k0-q0)_m + p - j| <= W (duplicated per head)
    deltas = [0, -W, -(2 * W)]
    p = np.arange(KW)[:, None]
    j = np.arange(QT)[None, :]
    m = np.stack([(np.abs(d + p - j) <= W) for d in deltas], axis=1)  # [128,3,114]
    m = np.repeat(m[:, :, None, :], HPC, axis=2)  # [128, 3, 2, 114]
    return np.ascontiguousarray(m.astype(ml_dtypes.bfloat16))


def _build_program():
    nc = bacc.Bacc("TRN2", target_bir_lowering=False, debug=False, enable_partition_id=False, enable_asserts=False, monotonic_sem_count=0)

    qh_d = nc.dram_tensor("qh", [128, L], BF16, kind="ExternalInput")
    kh_d = nc.dram_tensor("kh", [128, L], BF16, kind="ExternalInput")
    vw_d = nc.dram_tensor("vw", [128, HPC, NT, E + 1], BF16, kind="ExternalInput")
    mk_d = nc.dram_tensor("mk", [128, 3, HPC, QT], BF16, kind="ExternalInput")
    out_d = nc.dram_tensor("o", [HPC, L, E], F32, kind="ExternalOutput")

    with tile.TileContext(nc) as tc:
        with (
            tc.tile_pool(name="const", bufs=1) as cpool,
            tc.tile_pool(name="work", bufs=4) as work,
            tc.tile_pool(name="ps", bufs=2, space="PSUM") as ps,
            tc.tile_pool(name="ps1", bufs=2, space="PSUM") as ps1,
        ):
            qh_s = cpool.tile([128, L], BF16)
            kh_s = cpool.tile([128, L], BF16)
            vw_s = cpool.tile([128, HPC, NT, E + 1], BF16)
            mk_s = cpool.tile([128, 3, HPC, QT], BF16)
            nc.gpsimd.dma_start(mk_s[:], mk_d.ap()[:])
            nc.gpsimd.dma_start(vw_s[:, :, 0:3, :], vw_d.ap()[:, :, 0:3, :])
            nc.gpsimd.dma_start(vw_s[:, :, 3:NT, :], vw_d.ap()[:, :, 3:NT, :])
            # small first slices so tile 0/1 deps land early, then the rest
            FS = 256
            nc.sync.dma_start(kh_s[:, 0:FS], kh_d.ap()[:, 0:FS])
            nc.sync.dma_start(qh_s[:, 0:FS], qh_d.ap()[:, 0:FS])
            nc.sync.dma_start(kh_s[:, FS:L], kh_d.ap()[:, FS:L])
            nc.sync.dma_start(qh_s[:, FS:L], qh_d.ap()[:, FS:L])
            obuf = cpool.tile([QT, HPC, NT - 1, E], F32)
            # warm the exp table while DMAs stream
            dum = work.tile([1, 1], F32, tag="dum")
            nc.scalar.activation(dum[:], dum[:], EXP)

            for t in range(NT):
                q0, k0, mid, so = _PARAMS[t]
                kwin = slice(k0, k0 + KW)
                qwin = slice(q0, q0 + QT)
                # scores^T per head into adjacent PSUM banks
                st = ps.tile([KW, HPC, PSB], F32, tag="st")
                for h in range(HPC):
                    hp = h * E
                    nc.tensor.matmul(
                        st[:, h, 0:QT],
                        kh_s[hp : hp + E, kwin],
                        qh_s[hp : hp + E, qwin],
                    )
                # exp(scores/8), both heads in one op
                ex = work.tile([KW, HPC, QT], BF16, tag="ex")
                nc.scalar.activation(ex[:], st[:, :, 0:QT], EXP, scale=1.0 / 8.0)
                # band mask (0/1 multiply), both heads (mask identical per head)
                at = work.tile([KW, HPC, QT], BF16, tag="at")
                nc.vector.tensor_mul(at[:], ex[:], mk_s[:, mid, :, :])
                # attn^T @ [V_win | 1] -> [q, 65]: cols 0..63 out, col 64 denom
                o = ps1.tile([QT, HPC, PSB], F32, tag="o")
                for h in range(HPC):
                    nc.tensor.matmul(o[:, h, 0 : E + 1], at[:, h, :], vw_s[:, h, t, :])
                # normalize: out[q, e] = o[q, e] / o[q, 64], both heads at once
                rc = work.tile([QT, HPC], F32, tag="rc")
                nc.vector.reciprocal(rc[:], o[:, :, E])
                if t < NT - 1:
                    nc.vector.tensor_tensor(
                        obuf[:, :, t, :],
                        o[:, :, 0:E],
                        rc[:].broadcast_to([QT, HPC, E]),
                        mybir.AluOpType.mult,
                    )
                else:
                    for h in range(HPC):
                        oo = work.tile([QT, E], F32, tag=f"oo{h}")
                        nc.vector.tensor_scalar_mul(
                            oo[:], o[:, h, 0:E], rc[:, h : h + 1]
                        )
                        nc.sync.dma_start(
                            out_d.ap()[h, q0 + so : L, :], oo[so:QT, :]
                        )
                if t in (3, 7, 11, 14, 16):
                    c0 = {3: 0, 7: 4, 11: 8, 14: 12, 16: 15}[t]
                    c1 = t + 1
                    for h in range(HPC):
                        eng = nc.sync
                        eng.dma_start(
                            out_d.ap()[h, c0 * QT : c1 * QT, :].rearrange(
                                "(t p) e -> p t e", p=QT
                            ),
                            obuf[:, h, c0:c1, :],
                        )

    nc.compile()
    return nc


_NC_CACHE = None


def _get_program():
    global _NC_CACHE
    if _NC_CACHE is None:
        _NC_CACHE = _build_program()
    return _NC_CACHE


def _core_inputs(queries, keys, values, c, masks):
    bf = ml_dtypes.bfloat16
    qt = np.empty((128, L), dtype=np.float32)
    kt = np.empty((128, L), dtype=np.float32)
    vw = np.ones((128, HPC, NT, E + 1), dtype=bf)
    k0s = np.array([p[1] for p in _PARAMS])  # [NT]
    rows = k0s[:, None] + np.arange(KW)[None, :]  # [NT, 128]
    for j in range(HPC):
        u = HPC * c + j
        b, h = divmod(u, H)
        qt[E * j : E * (j + 1)] = queries[b, :, h, :].T
        kt[E * j : E * (j + 1)] = keys[b, :, h, :].T
        vh = values[b, :, h, :]  # [L, E]
        vw[:, j, :, :E] = vh[rows].transpose(1, 0, 2).astype(bf)
    return {
        "qh": qt.astype(bf),
        "kh": kt.astype(bf),
        "vw": vw,
        "mk": masks,
    }


def _run(queries, keys, values, trace=False):
    nc = _get_program()
    masks = _build_masks()
    in_maps = [
        _core_inputs(queries, keys, values, c, masks) for c in range(NCORES)
    ]
    res = run_bass_kernel_spmd(nc, in_maps, list(range(NCORES)), trace=trace)
    out = np.empty((B, L, H, E), dtype=np.float32)
    for c in range(NCORES):
        o = res.results[c]["o"]
        for j in range(HPC):
            u = HPC * c + j
            b, h = divmod(u, H)
            out[b, :, h, :] = o[j]
    return out, res


def kernel(queries, keys, values):
    out, _ = _run(
        np.asarray(queries, dtype=np.float32),
        np.asarray(keys, dtype=np.float32),
        np.asarray(values, dtype=np.float32),
    )
    return out


# revision 22
# speedup vs baseline: 1.0719x; 1.0318x over previous
"""Banded local attention on 8 Trainium2 NeuronCores (Bass/Tile).

Problem: B=2, L=2048, H=8, E=64, band |i-j| <= w with w = ceil(1.2*log2(L)/2) = 7.

Sharding: 16 (batch, head) units across 8 cores, 2 units per core.
Each core computes its two heads' banded attention fully independently.

Per-head algorithm (18 query tiles of 114 queries):
  For query tile [q0, q0+114) the band only touches keys [q0-7, q0+121), which
  fits a single 128-key window [k0, k0+128).  Scores are computed transposed,
  ST[k, q] = K_win @ Q_tile^T, in bf16 with e on partitions; the K=64
  contraction means the two heads (SBUF partition bases 0 and 64) run on
  disjoint PE row-groups and pipeline tightly.  exp(ST/8) on ScalarE (no max
  subtraction: unit-scale inputs can't overflow exp; softmax is
  shift-invariant).  Multiply by the 0/1 band mask (out-of-band -> exactly 0,
  matching exp(-inf)).  One matmul with V_aug = [V_win | 1] as stationary gives
  OT[65, q] = [unnormalized out^T; denominator row].  PE-transpose OT,
  reciprocal of the denominator column and a per-partition tensor_scalar
  multiply produce the normalized [q, 64] output tile, accumulated in an SBUF
  buffer and stored with one large DMA per head.  Both heads share every
  elementwise op: their PSUM tiles live in adjacent banks of one two-bank tile
  ([*, 2, 512] fp32), so ScalarE/VectorE see a single strided AP.
"""

import ml_dtypes
import numpy as np

import concourse.bass as bass
import concourse.tile as tile
from concourse import bacc, mybir
from concourse.bass_utils import run_bass_kernel_spmd

B, L, H, E = 2, 2048, 8, 64
W = 7
NCORES = 8
QT = 114  # queries per tile
KW = 128  # key window per tile
NT = 18  # tiles per head
HPC = 2  # heads (b,h units) per core
PSB = 512  # fp32 elements per PSUM bank
F32 = mybir.dt.float32
BF16 = mybir.dt.bfloat16

EXP = mybir.ActivationFunctionType.Exp


def _tile_params():
    params = []
    for t in range(NT):
        q0 = t * QT if t < NT - 1 else L - QT
        if t == 0:
            k0 = 0
        elif t < NT - 1:
            k0 = t * QT - W
        else:
            k0 = L - KW
        mid = 0 if t == 0 else (1 if t < NT - 1 else 2)
        so = 0 if t < NT - 1 else (NT - 1) * QT - q0  # rows already stored by t-1
        params.append((q0, k0, mid, so))
    return params


_PARAMS = _tile_params()


def _build_masks():
    # mask[p, m, h, j] = 1.0 iff |(k0-q0)_m + p - j| <= W (duplicated per head)
    deltas = [0, -W, -(2 * W)]
    p = np.arange(KW)[:, None]
    j = np.arange(QT)[None, :]
    m = np.stack([(np.abs(d + p - j) <= W) for d in deltas], axis=1)  # [128,3,114]
    m = np.repeat(m[:, :, None, :], HPC, axis=2)  # [128, 3, 2, 114]
    return np.ascontiguousarray(m.astype(ml_dtypes.bfloat16))


def _build_program():
    nc = bacc.Bacc("TRN2", target_bir_lowering=False, debug=False, enable_partition_id=False, enable_asserts=False, monotonic_sem_count=0)

    qh_d = nc.dram_tensor("qh", [128, L], BF16, kind="ExternalInput")
    kh_d = nc.dram_tensor("kh", [128, L], BF16, kind="ExternalInput")
    vw_d = nc.dram_tensor("vw", [128, HPC, NT, E + 1], BF16, kind="ExternalInput")
    # fs = [kh[:, :FS] | qh[:, :FS] | masks | vw[:, :, :3, :]] packed: one DMA
    # (one completion receipt) covers every dependency of the first tiles
    FS = 256
    FSW = 2 * FS + 3 * HPC * QT + HPC * 3 * (E + 1)
    fs_d = nc.dram_tensor("fs", [128, FSW], BF16, kind="ExternalInput")
    out_d = nc.dram_tensor("o", [HPC, L, E], F32, kind="ExternalOutput")

    with tile.TileContext(nc) as tc:
        with (
            tc.tile_pool(name="const", bufs=1) as cpool,
            tc.tile_pool(name="work", bufs=4) as work,
            tc.tile_pool(name="ps", bufs=2, space="PSUM") as ps,
            tc.tile_pool(name="ps1", bufs=2, space="PSUM") as ps1,
        ):
            qh_s = cpool.tile([128, L], BF16)
            kh_s = cpool.tile([128, L], BF16)
            vw_s = cpool.tile([128, HPC, NT, E + 1], BF16)
            fs_s = cpool.tile([128, FSW], BF16)
            nc.sync.dma_start(fs_s[:], fs_d.ap()[:])
            nc.sync.dma_start(kh_s[:], kh_d.ap()[:])
            nc.sync.dma_start(qh_s[:], qh_d.ap()[:])
            nc.gpsimd.dma_start(vw_s[:], vw_d.ap()[:])
            kh0 = fs_s[:, 0:FS]
            qh0 = fs_s[:, FS : 2 * FS]
            mk_s = fs_s[:, 2 * FS : 2 * FS + 3 * HPC * QT].rearrange(
                "p (m h q) -> p m h q", m=3, h=HPC
            )
            vw0 = fs_s[:, 2 * FS + 3 * HPC * QT : FSW].rearrange(
                "p (h t e) -> p h t e", h=HPC, t=3
            )
            obuf = cpool.tile([QT, HPC, NT - 1, E], F32)
            # warm the exp table while DMAs stream
            dum = work.tile([1, 1], F32, tag="dum")
            nc.scalar.activation(dum[:], dum[:], EXP)

            order = list(range(0, 9)) + [17] + list(range(9, 17))
            for t in order:
                q0, k0, mid, so = _PARAMS[t]
                early = t < 2
                ksrc = kh0 if early else kh_s
                qsrc = qh0 if early else qh_s
                # scores^T per head into adjacent PSUM banks
                st = ps.tile([KW, HPC, PSB], F32, tag="st")
                for h in range(HPC):
                    hp = h * E
                    nc.tensor.matmul(
                        st[:, h, 0:QT],
                        ksrc[hp : hp + E, k0 : k0 + KW],
                        qsrc[hp : hp + E, q0 : q0 + QT],
                    )
                # exp(scores/8), both heads in one op
                ex = work.tile([KW, HPC, QT], BF16, tag="ex")
                nc.scalar.activation(ex[:], st[:, :, 0:QT], EXP, scale=1.0 / 8.0)
                # band mask (0/1 multiply), both heads (mask identical per head)
                at = work.tile([KW, HPC, QT], BF16, tag="at")
                nc.gpsimd.tensor_tensor(at[:], ex[:], mk_s[:, mid, :, :], mybir.AluOpType.mult)
                # attn^T @ [V_win | 1] -> [q, 65]: cols 0..63 out, col 64 denom
                o = ps1.tile([QT, HPC, PSB], F32, tag="o")
                for h in range(HPC):
                    vsrc = vw0 if t < 3 else vw_s
                    nc.tensor.matmul(o[:, h, 0 : E + 1], at[:, h, :], vsrc[:, h, t, :])
                # normalize: out[q, e] = o[q, e] / o[q, 64], both heads at once
                rc = work.tile([QT, HPC], F32, tag="rc")
                nc.vector.reciprocal(rc[:], o[:, :, E])
                if t < NT - 1:
                    nc.vector.tensor_tensor(
                        obuf[:, :, t, :],
                        o[:, :, 0:E],
                        rc[:].broadcast_to([QT, HPC, E]),
                        mybir.AluOpType.mult,
                    )
                else:
                    for h in range(HPC):
                        oo = work.tile([QT, E], F32, tag=f"oo{h}")
                        nc.vector.tensor_scalar_mul(
                            oo[:], o[:, h, 0:E], rc[:, h : h + 1]
                        )
                        nc.sync.dma_start(
                            out_d.ap()[h, q0 + so : L, :], oo[so:QT, :]
                        )
                if t in (3, 7, 11, 14, 16):
                    c0 = {3: 0, 7: 4, 11: 8, 14: 12, 16: 15}[t]
                    c1 = t + 1
                    for h in range(HPC):
                        eng = nc.sync
                        eng.dma_start(
                            out_d.ap()[h, c0 * QT : c1 * QT, :].rearrange(
                                "(t p) e -> p t e", p=QT
                            ),
                            obuf[:, h, c0:c1, :],
                        )

    nc.compile()
    return nc


_NC_CACHE = None


def _get_program():
    global _NC_CACHE
    if _NC_CACHE is None:
        _NC_CACHE = _build_program()
    return _NC_CACHE


def _core_inputs(queries, keys, values, c, masks):
    bf = ml_dtypes.bfloat16
    qt = np.empty((128, L), dtype=np.float32)
    kt = np.empty((128, L), dtype=np.float32)
    vw = np.ones((128, HPC, NT, E + 1), dtype=bf)
    k0s = np.array([p[1] for p in _PARAMS])  # [NT]
    rows = k0s[:, None] + np.arange(KW)[None, :]  # [NT, 128]
    for j in range(HPC):
        u = HPC * c + j
        b, h = divmod(u, H)
        qt[E * j : E * (j + 1)] = queries[b, :, h, :].T
        kt[E * j : E * (j + 1)] = keys[b, :, h, :].T
        vh = values[b, :, h, :]  # [L, E]
        vw[:, j, :, :E] = vh[rows].transpose(1, 0, 2).astype(bf)
    qh = qt.astype(bf)
    kh = kt.astype(bf)
    fs = np.concatenate(
        [
            kh[:, :256],
            qh[:, :256],
            masks.reshape(128, -1),
            vw[:, :, :3, :].reshape(128, -1),
        ],
        axis=1,
    )
    return {
        "qh": qh,
        "kh": kh,
        "vw": vw,
        "fs": np.ascontiguousarray(fs),
    }


def _run(queries, keys, values, trace=False):
    nc = _get_program()
    masks = _build_masks()
    in_maps = [
        _core_inputs(queries, keys, values, c, masks) for c in range(NCORES)
    ]
    res = run_bass_kernel_spmd(nc, in_maps, list(range(NCORES)), trace=trace)
    out = np.empty((B, L, H, E), dtype=np.float32)
    for c in range(NCORES):
        o = res.results[c]["o"]
        for j in range(HPC):
            u = HPC * c + j
            b, h = divmod(u, H)
            out[b, :, h, :] = o[j]
    return out, res


def kernel(queries, keys, values):
    out, _ = _run(
        np.asarray(queries, dtype=np.float32),
        np.asarray(keys, dtype=np.float32),
        np.asarray(values, dtype=np.float32),
    )
    return out
